# revision 1
# baseline (speedup 1.0000x reference)
"""Distributed GINE GNN kernel for 8 Trainium2 NeuronCores.

Sharding: nodes partitioned contiguously across cores (12500/core, padded to
12544 = 98 windows of 128); edges assigned to the core owning their dst;
src features read from a replicated bf16 copy of h, AllGather'd per layer.

Edges are bucketed host-side by dst window (128 nodes). Per 128-edge chunk:
  e   = [attr|1] @ [We;be]          (PE, K=17)
  e  += h_full[src]                 (gather via indirect DMA + PE identity-matmul
                                     accumulate into the e PSUM bank)
  m   = relu(e)                     (ACT: fused relu on PSUM->SBUF eviction)
  agg[dst] += m                     (PE: one-hot matmul, accumulated in PSUM
                                     over all chunks of the window)
One-hot tiles are built on DVE via tensor_scalar is_equal against an iota row.
Node MLP runs D-major (BN folded into W1); PE transposes convert between
node-major and D-major. Pooling = one-hot matmuls by graph id (fp32 PSUM),
assembled into a global buffer by indirect row scatter, AllReduce'd; the
classifier runs redundantly on every core.
"""

import numpy as np
import ml_dtypes

import concourse.bass as bass
import concourse.bacc as bacc
import concourse.mybir as mybir
import concourse.tile as tile
from concourse import bass_utils

# ---------------- problem constants ----------------
N = 100000
E = 1600000
D = 128
ED = 16
L = 3
G = 1024
C = 10
BN_EPS = 1e-5

CORES = 8
NPC = N // CORES          # 12500
NLOC = 12544              # 98 * 128
NW = NLOC // 128          # 98 dst windows
HROWS = CORES * NLOC      # 100352 rows of replicated h

GLW = 256                 # local graph-id window span (2 psum tiles of 128)

BF16 = mybir.dt.bfloat16
F32 = mybir.dt.float32
I16 = mybir.dt.int16
I32 = mybir.dt.int32

bf16 = ml_dtypes.bfloat16


# ---------------- host-side prep ----------------

def prepare(x, edge_attr, We, be, W1, b1, gamma, beta, W2, b2,
            Wc1, bc1, Wc2, bc2, edge_index, batch):
    x = np.asarray(x, np.float32)
    edge_attr = np.asarray(edge_attr, np.float32)
    edge_index = np.asarray(edge_index, np.int64)
    batch = np.asarray(batch, np.int64)

    rstd = 1.0 / np.sqrt(1.0 + BN_EPS)
    s = rstd * np.asarray(gamma, np.float32)
    W1f = np.asarray(W1, np.float32) * s[:, None, :]
    b1f = np.asarray(b1, np.float32) * s + np.asarray(beta, np.float32)

    src, dst = edge_index[0], edge_index[1]
    sc = src // NPC
    src_row = (sc * NLOC + (src - sc * NPC)).astype(np.int32)
    core_of_edge = dst // NPC
    dst_local = dst - core_of_edge * NPC

    # bucket edges by (core, dst window); SPW = max bucket size rounded to 128
    key = core_of_edge * NW + dst_local // 128
    order = np.argsort(key, kind="stable")
    key_s = key[order]
    bounds = np.searchsorted(key_s, np.arange(CORES * NW + 1))
    SPW = int((np.diff(bounds).max() + 127) // 128 * 128)
    EC = NW * SPW
    NCH = EC // 128                      # chunks per core

    g_rows = np.zeros((CORES, EC), np.int32)
    dstrel = np.full((CORES, EC), -1, np.int32)
    attr_slots = np.zeros((CORES, EC, ED), np.float32)
    for c in range(CORES):
        for w in range(NW):
            lo, hi = bounds[c * NW + w], bounds[c * NW + w + 1]
            eids = order[lo:hi]
            base = w * SPW
            n = hi - lo
            g_rows[c, base:base + n] = src_row[eids]
            dstrel[c, base:base + n] = dst_local[eids] % 128
            attr_slots[c, base:base + n] = edge_attr[eids]

    # slot s holds edge at (partition s%128, chunk s//128)
    def wrap_cols(a):  # [CORES, EC] -> [CORES, 128, NCH]
        return np.transpose(a.reshape(CORES, NCH, 128), (0, 2, 1))

    g_off = np.ascontiguousarray(wrap_cols(g_rows))
    dstrel_w = np.ascontiguousarray(wrap_cols(dstrel)).astype(np.float32)

    attrT = np.ones((CORES, ED + 1, EC), np.float32)
    attrT[:, :ED, :] = np.transpose(attr_slots, (0, 2, 1))
    attrT = attrT.astype(bf16)

    # x: replicated storage rows + per-core node-major wrapped local panel
    x_full = np.zeros((HROWS, D), np.float32)
    x_locN = np.zeros((CORES, 128, NLOC), np.float32)
    for c in range(CORES):
        xc = x[c * NPC:(c + 1) * NPC]
        x_full[c * NLOC: c * NLOC + NPC] = xc
        xp = np.zeros((NLOC, D), np.float32)
        xp[:NPC] = xc
        # node i at partition i%128, cols (i//128)*128 : +128
        x_locN[c] = xp.reshape(NW, 128, D).transpose(1, 0, 2).reshape(128, NLOC)
    x_full = x_full.astype(bf16)
    x_locN = x_locN.astype(bf16)

    # pooling: glocal[p, w] = batch[local node w*128+p] - gbase (pad -> -1)
    glocal = np.full((CORES, 128, NW), -1, np.float32)
    pool_rows = np.zeros((CORES, 128, 2), np.int32)
    for c in range(CORES):
        bb = batch[c * NPC:(c + 1) * NPC]
        gb = int(bb[0])
        span = int(bb[-1] - bb[0])
        assert span < GLW, f"graph span {span} exceeds {GLW}"
        gl = np.full(NLOC, -1, np.int64)
        gl[:NPC] = bb - gb
        glocal[c] = gl.reshape(NW, 128).T.astype(np.float32)
        pool_rows[c, :, 0] = gb + np.arange(128)
        pool_rows[c, :, 1] = gb + 128 + np.arange(128)
    pool_rows = np.clip(pool_rows, 0, G + GLW - 1).astype(np.int32)

    weights = dict(
        WeT=np.ascontiguousarray(np.asarray(We, np.float32)).astype(bf16),
        beb=np.asarray(be, np.float32).astype(bf16),
        W1f=W1f.astype(bf16), W2=np.asarray(W2, np.float32).astype(bf16),
        b1f=b1f.astype(np.float32), b2=np.asarray(b2, np.float32),
        Wc1=np.asarray(Wc1, np.float32).astype(bf16),
        Wc2=np.asarray(Wc2, np.float32).astype(bf16),
        bc1=np.asarray(bc1, np.float32), bc2=np.asarray(bc2, np.float32),
    )
    aux = dict(
        iota=np.tile(np.arange(128, dtype=np.float32), (128, 1)),
        iota2=np.tile(np.arange(GLW, dtype=np.float32), (128, 1)),
        ident=np.eye(128, dtype=np.float32).astype(bf16),
    )
    return dict(SPW=SPW, EC=EC, g_off=g_off, dstrel=dstrel_w, attrT=attrT,
                x_full=x_full, x_locN=x_locN, glocal=glocal,
                pool_rows=pool_rows, weights=weights, aux=aux)


# ---------------- device program ----------------

def build_program(SPW):
    nc = bacc.Bacc("TRN2", target_bir_lowering=False, debug=False,
                   num_devices=CORES, num_swdge_queues=4)
    EC = NW * SPW
    NCH = EC // 128
    CPW = SPW // 128                    # chunks per window
    SUB = 4                             # chunks per psum e-bank

    t_xfull = nc.dram_tensor("x_full", [HROWS, D], BF16, kind="ExternalInput")
    t_xlocN = nc.dram_tensor("x_locN", [128, NLOC], BF16, kind="ExternalInput")
    t_goff = nc.dram_tensor("g_off", [128, NCH], I32, kind="ExternalInput")
    t_dstrel = nc.dram_tensor("dstrel", [128, NCH], F32, kind="ExternalInput")
    t_attrT = nc.dram_tensor("attrT", [ED + 1, EC], BF16, kind="ExternalInput")
    t_glocal = nc.dram_tensor("glocal", [128, NW], F32, kind="ExternalInput")
    t_prows = nc.dram_tensor("pool_rows", [128, 2], I32, kind="ExternalInput")
    t_iota = nc.dram_tensor("iota", [128, 128], F32, kind="ExternalInput")
    t_iota2 = nc.dram_tensor("iota2", [128, GLW], F32, kind="ExternalInput")
    t_ident = nc.dram_tensor("ident", [128, 128], BF16, kind="ExternalInput")
    t_WeT = nc.dram_tensor("WeT", [L, ED, D], BF16, kind="ExternalInput")
    t_beb = nc.dram_tensor("beb", [L, D], BF16, kind="ExternalInput")
    t_W1f = nc.dram_tensor("W1f", [L, D, D], BF16, kind="ExternalInput")
    t_W2 = nc.dram_tensor("W2", [L, D, D], BF16, kind="ExternalInput")
    t_b1f = nc.dram_tensor("b1f", [L, D], F32, kind="ExternalInput")
    t_b2 = nc.dram_tensor("b2", [L, D], F32, kind="ExternalInput")
    t_Wc1 = nc.dram_tensor("Wc1", [D, D], BF16, kind="ExternalInput")
    t_Wc2 = nc.dram_tensor("Wc2", [D, C], BF16, kind="ExternalInput")
    t_bc1 = nc.dram_tensor("bc1", [D], F32, kind="ExternalInput")
    t_bc2 = nc.dram_tensor("bc2", [C], F32, kind="ExternalInput")

    t_out = nc.dram_tensor("out", [C, G], F32, kind="ExternalOutput")

    GROWS = G + GLW

    with tile.TileContext(nc) as tc:
        with (
            tc.tile_pool(name="persist", bufs=1) as pp,
            tc.tile_pool(name="edges", bufs=3) as ep,
            tc.tile_pool(name="small", bufs=4) as sp,
            tc.tile_pool(name="nodes", bufs=1) as np1,
            tc.tile_pool(name="pse", bufs=2, space="PSUM") as pse,
            tc.tile_pool(name="psa", bufs=2, space="PSUM") as psa,
            tc.tile_pool(name="psn", bufs=2, space="PSUM") as psn,
            tc.tile_pool(name="dram", bufs=1, space="DRAM") as dp,
        ):
            # ---- persistent tiles ----
            h_pan = pp.tile([128, NLOC], BF16, tag="h")          # node-major
            nc.sync.dma_start(h_pan[:], t_xlocN[:])
            goff_t = pp.tile([128, NCH], I32, tag="goff")
            nc.sync.dma_start(goff_t[:], t_goff[:])
            dstrel_t = pp.tile([128, NCH], F32, tag="dstrel")
            nc.sync.dma_start(dstrel_t[:], t_dstrel[:])
            glocal_t = pp.tile([128, NW], F32, tag="glocal")
            nc.sync.dma_start(glocal_t[:], t_glocal[:])
            prows_t = pp.tile([128, 2], I32, tag="prows")
            nc.sync.dma_start(prows_t[:], t_prows[:])
            iota_t = pp.tile([128, 128], F32, tag="iota")
            nc.sync.dma_start(iota_t[:], t_iota[:])
            iota2_t = pp.tile([128, GLW], F32, tag="iota2")
            nc.sync.dma_start(iota2_t[:], t_iota2[:])
            ident = pp.tile([128, 128], BF16, tag="ident")
            nc.sync.dma_start(ident[:], t_ident[:])

            biases = pp.tile([128, 8], F32, tag="biases")
            for l in range(L):
                nc.sync.dma_start(biases[:, 2 * l:2 * l + 1], t_b1f[l, :, None])
                nc.sync.dma_start(biases[:, 2 * l + 1:2 * l + 2], t_b2[l, :, None])
            nc.sync.dma_start(biases[:, 6:7], t_bc1[:, None])
            nc.sync.dma_start(biases[:C, 7:8], t_bc2[:, None])

            WCOLS = 3 * L * D + D + C
            wts = pp.tile([128, WCOLS], BF16, tag="wts")
            nc.vector.memset(wts[:], 0.0)
            for l in range(L):
                nc.sync.dma_start(wts[:ED, 3 * l * D:3 * l * D + D], t_WeT[l])
                nc.sync.dma_start(wts[ED:ED + 1, 3 * l * D:3 * l * D + D],
                                  t_beb[l, None, :])
                nc.sync.dma_start(wts[:, 3 * l * D + D:3 * l * D + 2 * D], t_W1f[l])
                nc.sync.dma_start(wts[:, 3 * l * D + 2 * D:3 * l * D + 3 * D], t_W2[l])
            nc.sync.dma_start(wts[:, 3 * L * D:3 * L * D + D], t_Wc1[:])
            nc.sync.dma_start(wts[:, 3 * L * D + D:3 * L * D + D + C], t_Wc2[:])

            agg = pp.tile([128, NLOC], BF16, tag="agg")          # node-major

            ag_in = dp.tile([NLOC, D], BF16, tag="ag_in")
            ag_out = []
            for i in range(2):
                ago = dp.tile([HROWS, D], BF16, tag=f"ag_out{i}", name=f"ag_out{i}")
                ag_out.append(ago)
            pool_dram = dp.tile([GROWS, D], F32, tag="pool_dram")
            pool_red = dp.tile([GROWS, D], F32, tag="pool_red")

            def edge_layer(l, h_src_dram):
                wcol = 3 * l * D
                for w in range(NW):
                    # gather h[src] for this window's chunks
                    hg = ep.tile([128, CPW, D], BF16, tag="hg")
                    for j in range(CPW):
                        ch = w * CPW + j
                        bi = nc.gpsimd.indirect_dma_start(
                            out=hg[:, j, :], out_offset=None, in_=h_src_dram,
                            in_offset=bass.IndirectOffsetOnAxis(
                                ap=goff_t[:, ch:ch + 1], axis=0))
                        if j % 4:
                            bi.ins.queue = f"qPoolDynamic{j % 4}"
                    at = ep.tile([ED + 1, SPW], BF16, tag="attrT")
                    nc.sync.dma_start(at[:], t_attrT[:, w * SPW:(w + 1) * SPW])
                    m = ep.tile([128, CPW, D], BF16, tag="m")
                    apsum = psa.tile([128, D], F32, tag="apsum")
                    for b in range((CPW + SUB - 1) // SUB):
                        jhi = min((b + 1) * SUB, CPW)
                        nj = jhi - b * SUB
                        eps = pse.tile([128, SUB * D], F32, tag="eps")
                        for j in range(b * SUB, jhi):
                            nc.tensor.matmul(
                                eps[:, (j - b * SUB) * D:(j - b * SUB + 1) * D],
                                at[:, j * 128:(j + 1) * 128],
                                wts[:ED + 1, wcol:wcol + D],
                                start=(j == b * SUB), stop=False)
                        # accumulate gathered h into the same psum columns
                        for j in range(b * SUB, jhi):
                            nc.tensor.matmul(
                                eps[:, (j - b * SUB) * D:(j - b * SUB + 1) * D],
                                ident[:], hg[:, j, :],
                                start=False, stop=(j == jhi - 1))
                        # m = relu(e + h), psum -> sbuf
                        nc.scalar.activation(
                            m[:, b * SUB:jhi, :].rearrange("p a d -> p (a d)"),
                            eps[:, :nj * D],
                            mybir.ActivationFunctionType.Relu)
                    # scatter: agg_psum += onehot.T @ m, chunk by chunk
                    for j in range(CPW):
                        ch = w * CPW + j
                        oh = sp.tile([128, 128], BF16, tag="oh")
                        nc.vector.tensor_scalar(
                            oh[:], iota_t[:], dstrel_t[:, ch:ch + 1], None,
                            op0=mybir.AluOpType.is_equal)
                        nc.tensor.matmul(
                            apsum[:], oh[:], m[:, j, :],
                            start=(j == 0), stop=(j == CPW - 1))
                    nc.scalar.copy(agg[:, w * 128:(w + 1) * 128], apsum[:])

            def node_mlp(l):
                zN = np1.tile([128, NLOC], BF16, tag="zN")
                nc.vector.tensor_add(zN[:], h_pan[:], agg[:])
                zD = np1.tile([128, NLOC], BF16, tag="zD")
                for w in range(NW):
                    tp = psn.tile([128, 128], BF16, tag="nps", name="tp")
                    nc.tensor.transpose(tp[:], zN[:, w * 128:(w + 1) * 128], ident[:])
                    nc.vector.tensor_copy(zD[:, w * 128:(w + 1) * 128], tp[:])
                spans = [(i * 512, 512) for i in range(NLOC // 512)]
                if NLOC % 512:
                    spans.append((NLOC - NLOC % 512, NLOC % 512))
                for (o, wd) in spans:
                    ps = psn.tile([128, 512], F32, tag="nps")
                    nc.tensor.matmul(ps[:, :wd],
                                     wts[:, 3 * l * D + D:3 * l * D + 2 * D],
                                     zD[:, o:o + wd], start=True, stop=True)
                    nc.scalar.activation(zD[:, o:o + wd], ps[:, :wd],
                                         mybir.ActivationFunctionType.Relu,
                                         bias=biases[:, 2 * l:2 * l + 1])
                for (o, wd) in spans:
                    ps = psn.tile([128, 512], F32, tag="nps")
                    nc.tensor.matmul(ps[:, :wd],
                                     wts[:, 3 * l * D + 2 * D:3 * l * D + 3 * D],
                                     zD[:, o:o + wd], start=True, stop=True)
                    nc.scalar.activation(zD[:, o:o + wd], ps[:, :wd],
                                         mybir.ActivationFunctionType.Relu,
                                         bias=biases[:, 2 * l + 1:2 * l + 2])
                # back to node-major h
                for w in range(NW):
                    tp = psn.tile([128, 128], BF16, tag="nps", name="tp")
                    nc.tensor.transpose(tp[:], zD[:, w * 128:(w + 1) * 128], ident[:])
                    nc.vector.tensor_copy(h_pan[:, w * 128:(w + 1) * 128], tp[:])

            # ---------------- layers ----------------
            for l in range(L):
                h_src = t_xfull[:] if l == 0 else ag_out[(l - 1) % 2][:]
                edge_layer(l, h_src)
                node_mlp(l)
                if l < L - 1:
                    nc.sync.dma_start(
                        ag_in[:].rearrange("(b p) d -> p b d", p=128),
                        h_pan[:].rearrange("p (b d) -> p b d", d=D))
                    nc.gpsimd.collective_compute(
                        "AllGather", mybir.AluOpType.bypass,
                        ins=[ag_in.opt()], outs=[ag_out[l % 2].opt()],
                        replica_groups=[list(range(CORES))])

            # ---------------- pooling ----------------
            pps = psa.tile([128, 2, D], F32, tag="pps")
            pps0 = pps[:, 0, :]
            pps1 = pps[:, 1, :]
            for w in range(NW):
                oh0 = sp.tile([128, 128], BF16, tag="oh")
                nc.vector.tensor_scalar(
                    oh0[:], iota2_t[:, :128], glocal_t[:, w:w + 1], None,
                    op0=mybir.AluOpType.is_equal)
                nc.tensor.matmul(pps0, oh0[:], h_pan[:, w * 128:(w + 1) * 128],
                                 start=(w == 0), stop=False)
                oh1 = sp.tile([128, 128], BF16, tag="oh")
                nc.vector.tensor_scalar(
                    oh1[:], iota2_t[:, 128:], glocal_t[:, w:w + 1], None,
                    op0=mybir.AluOpType.is_equal)
                nc.tensor.matmul(pps1, oh1[:], h_pan[:, w * 128:(w + 1) * 128],
                                 start=False, stop=(w == NW - 1))
            pool_sb = np1.tile([128, 2, D], F32, tag="pool_sb")
            nc.scalar.copy(pool_sb[:, 0, :], pps0)
            nc.scalar.copy(pool_sb[:, 1, :], pps1)

            # zero the global pooled buffer, then place partials at gbase rows
            zt = np1.tile([128, (GROWS // 128) * D], F32, tag="zt")
            nc.vector.memset(zt[:], 0.0)
            nc.sync.dma_start(
                pool_dram[:].rearrange("(a p) d -> p a d", p=128),
                zt[:].rearrange("p (a d) -> p a d", d=D))
            for i in range(2):
                nc.gpsimd.indirect_dma_start(
                    out=pool_dram[:], out_offset=bass.IndirectOffsetOnAxis(
                        ap=prows_t[:, i:i + 1], axis=0),
                    in_=pool_sb[:, i, :], in_offset=None)
            nc.gpsimd.collective_compute(
                "AllReduce", mybir.AluOpType.add,
                ins=[pool_dram.opt()], outs=[pool_red.opt()],
                replica_groups=[list(range(CORES))])

            # ---------------- classifier ----------------
            prows_n = np1.tile([128, G // 128, D], F32, tag="prows_n")
            nc.sync.dma_start(
                prows_n[:], pool_red[:G, :].rearrange("(b p) d -> p b d", p=128))
            prows_bf = np1.tile([128, G // 128, D], BF16, tag="prows_bf")
            nc.vector.tensor_copy(
                prows_bf[:].rearrange("p a d -> p (a d)"),
                prows_n[:].rearrange("p a d -> p (a d)"))
            pooled_bf = np1.tile([128, G], BF16, tag="pooled_bf")   # D-major
            for b in range(G // 128):
                tp = psn.tile([128, 128], BF16, tag="nps", name="tp")
                nc.tensor.transpose(tp[:], prows_bf[:, b, :], ident[:])
                nc.vector.tensor_copy(pooled_bf[:, b * 128:(b + 1) * 128], tp[:])
            q1 = np1.tile([128, G], BF16, tag="q1")
            for o in range(0, G, 512):
                wd = min(512, G - o)
                ps = psn.tile([128, 512], F32, tag="nps")
                nc.tensor.matmul(ps[:, :wd], wts[:, 3 * L * D:3 * L * D + D],
                                 pooled_bf[:, o:o + wd], start=True, stop=True)
                nc.scalar.activation(q1[:, o:o + wd], ps[:, :wd],
                                     mybir.ActivationFunctionType.Relu,
                                     bias=biases[:, 6:7])
            outt = np1.tile([C, G], F32, tag="outt")
            for o in range(0, G, 512):
                wd = min(512, G - o)
                ps = psn.tile([128, 512], F32, tag="nps")
                nc.tensor.matmul(ps[:C, :wd], wts[:, 3 * L * D + D:3 * L * D + D + C],
                                 q1[:, o:o + wd], start=True, stop=True)
                nc.scalar.activation(outt[:, o:o + wd], ps[:C, :wd],
                                     mybir.ActivationFunctionType.Identity,
                                     bias=biases[:C, 7:8])
            nc.sync.dma_start(t_out[:], outt[:])

    nc.compile()
    return nc


_PROGRAM_CACHE = {}


def _get_program(SPW):
    if SPW not in _PROGRAM_CACHE:
        _PROGRAM_CACHE[SPW] = build_program(SPW)
    return _PROGRAM_CACHE[SPW]


def make_in_maps(prep):
    w = prep["weights"]
    a = prep["aux"]
    in_maps = []
    for c in range(CORES):
        in_maps.append({
            "x_full": prep["x_full"],
            "x_locN": np.ascontiguousarray(prep["x_locN"][c]),
            "g_off": np.ascontiguousarray(prep["g_off"][c]),
            "dstrel": np.ascontiguousarray(prep["dstrel"][c]),
            "attrT": np.ascontiguousarray(prep["attrT"][c]),
            "glocal": np.ascontiguousarray(prep["glocal"][c]),
            "pool_rows": np.ascontiguousarray(prep["pool_rows"][c]),
            "iota": a["iota"], "iota2": a["iota2"], "ident": a["ident"],
            "WeT": w["WeT"], "beb": w["beb"], "W1f": w["W1f"], "W2": w["W2"],
            "b1f": w["b1f"], "b2": w["b2"],
            "Wc1": w["Wc1"], "Wc2": w["Wc2"], "bc1": w["bc1"], "bc2": w["bc2"],
        })
    return in_maps


def postprocess(out):
    return np.ascontiguousarray(out.T.astype(np.float32))


def kernel(**inputs):
    prep = prepare(**{k: np.asarray(v) for k, v in inputs.items()})
    nc = _get_program(prep["SPW"])
    res = bass_utils.run_bass_kernel_spmd(nc, make_in_maps(prep),
                                          core_ids=list(range(CORES)))
    return postprocess(res.results[0]["out"])



# revision 36
# speedup vs baseline: 1.3679x; 1.3679x over previous
"""Distributed GINE GNN kernel for 8 Trainium2 NeuronCores.

Sharding: nodes partitioned contiguously across cores (12500/core, padded to
12544 = 98 windows of 128); edges assigned to the core owning their dst;
src features read from a replicated bf16 copy of h, AllGather'd per layer.

Edges are bucketed host-side by (dst window, src region). The gather of
h[src] uses bulk gpsimd.dma_gather (256B rows, int16 indices — hence 4
source regions of 25088 rows each), issued per (region, window-segment):
~44 large gathers per layer instead of ~1764 per-chunk indirect DMAs,
which removes the SWDGE descriptor-generation serialization on Pool.

Per 128-edge chunk:
  e   = [attr|1] @ [We;be]          (PE, K=17)
  e  += h_gathered                  (PE identity-matmul accumulate in PSUM)
  m   = relu(e)                     (ACT: fused relu on PSUM->SBUF eviction)
  agg[dst] += m                     (PE: one-hot matmul, accumulated in PSUM
                                     over all chunks of the window)
One-hot tiles are built on DVE via bf16 tensor_scalar is_equal (2x mode).
Node MLP runs D-major (BN folded into W1); PE transposes convert between
node-major and D-major. Pooling = one-hot matmuls by graph id (fp32 PSUM),
assembled into a global buffer by indirect row scatter, AllReduce'd; the
classifier runs redundantly on every core.
"""

import numpy as np
import ml_dtypes

import concourse.bass as bass
import concourse.bacc as bacc
import concourse.mybir as mybir
import concourse.tile as tile
from concourse import bass_utils

# ---------------------------------------------------------------------------
# Tile assigns the 8 DMASW completion-sem lanes round-robin in scheduled
# order, but each lane gets locked to the SWDGE queue of its first user, so
# multi-queue SWDGE DMAs (our 4-queue dma_gathers) trip "sem locked to other
# queue" depending on schedule order. Partition the lanes by queue instead
# (2 lanes per queue), which is also what the ring-reclaim protocol wants.
import concourse.tile_sem_assignment as _tsa


def _swdge_queue_of(inst):
    # must match bass_rust swdge_queue_num: only the custom SWDGE families
    # carry queue_num; InstDMACopy (indirect DMA) is always queue 0.
    qn = getattr(inst, "queue_num", None)
    return int(qn) if qn is not None else 0


_orig_assign_tick = _tsa.TileClockTick._assign_tick


def _assign_tick_qaware(self, inst):
    if (
        isinstance(inst, _tsa.DMAInst)
        and not isinstance(inst, _tsa.bass_isa.UserSyncedRemoteDMADescs)
        and inst.engine == mybir.EngineType.Pool
        and self.swdge_sem_count == 8
    ):
        qcnt = getattr(self, "_swdge_q_cnt", None)
        if qcnt is None:
            qcnt = self._swdge_q_cnt = [0, 0, 0, 0]
        q = _swdge_queue_of(inst)
        lane = 2 * q + (qcnt[q] % 2)
        qcnt[q] += 1
        self.next_sw_dma_idx = lane
        try:
            return _orig_assign_tick(self, inst)
        finally:
            pass
    return _orig_assign_tick(self, inst)


_tsa.TileClockTick._assign_tick = _assign_tick_qaware

# ---------------- problem constants ----------------
N = 100000
E = 1600000
D = 128
ED = 16
L = 3
G = 1024
C = 10
BN_EPS = 1e-5

CORES = 8
NPC = N // CORES          # 12500
NLOC = 12544              # 98 * 128
NW = NLOC // 128          # 98 dst windows
HROWS = CORES * NLOC      # 100352 rows of replicated h
NREG = 4                  # gather source regions (int16 idx limit)
QWIN = [25, 25, 24, 24]   # dst.. src windows per quarter-region
QSTART = [0, 25, 50, 74]
QEND = [24, 49, 73, 97]   # last window of each quarter (inclusive)
QN = [w * 128 for w in QWIN]           # local rows per quarter
QROWS = [CORES * n for n in QN]        # gathered rows per region (< 32768)
QOFF = [0, QROWS[0], QROWS[0] + QROWS[1], QROWS[0] + QROWS[1] + QROWS[2]]
SUB = 4                   # chunks per psum e-group
ASEG = 16                 # consumption chunks per attr tile
ACOLS = ASEG * 128        # cols per attr tile

GLW = 256                 # local graph-id window span (2 psum tiles of 128)

BF16 = mybir.dt.bfloat16
F32 = mybir.dt.float32
I16 = mybir.dt.int16
I32 = mybir.dt.int32

bf16 = ml_dtypes.bfloat16

DEBUG_NO_COLLECTIVES = False
DEBUG_NO_GATHER = False
DEBUG_NO_SCATTER = False
DEBUG_NO_EMM = False


# ---------------- host-side prep ----------------

def prepare(x, edge_attr, We, be, W1, b1, gamma, beta, W2, b2,
            Wc1, bc1, Wc2, bc2, edge_index, batch):
    x = np.asarray(x, np.float32)
    edge_attr = np.asarray(edge_attr, np.float32)
    edge_index = np.asarray(edge_index, np.int64)
    batch = np.asarray(batch, np.int64)

    rstd = 1.0 / np.sqrt(1.0 + BN_EPS)
    s = rstd * np.asarray(gamma, np.float32)
    W1f = np.asarray(W1, np.float32) * s[:, None, :]
    b1f = np.asarray(b1, np.float32) * s + np.asarray(beta, np.float32)

    src, dst = edge_index[0], edge_index[1]
    sc = src // NPC
    src_loc = src - sc * NPC                 # 0..12499
    src_w = src_loc // 128
    region = np.searchsorted(np.asarray(QEND), src_w, side="left").astype(np.int64)
    idx_in_reg = (sc * np.asarray(QN)[region]
                  + (src_loc - np.asarray(QSTART)[region] * 128)).astype(np.int64)
    core_of_edge = dst // NPC
    dst_local = dst - core_of_edge * NPC
    w_of_edge = dst_local // 128

    # sort edges by (core, window, region)
    key = (core_of_edge * NW + w_of_edge) * NREG + region
    order = np.argsort(key, kind="stable")
    key_s = key[order]
    bounds = np.searchsorted(key_s, np.arange(CORES * NW * NREG + 1))
    nwr = np.diff(bounds).reshape(CORES, NW, NREG)
    assert (nwr.sum(axis=2) > 0).all(), "empty dst window"
    CPR = int((nwr.max() + 127) // 128)      # chunks per (window, region)

    TOTCH = NW * NREG * CPR                  # consumption chunks per core
    RCH = NW * CPR                           # chunks per region stream
    GIDXC = TOTCH * 8                        # gidx cols (128 idx = 8 cols)
    NA = ((TOTCH + ASEG - 1) // ASEG)        # attr segments
    NA4 = NA * ACOLS

    # position of each edge within its (core, w, r) bucket
    pos_in_bucket = np.arange(E) - bounds[:-1][np.searchsorted(
        bounds, np.arange(E), side="right") - 1]
    # per-edge (sorted order) fields
    e_core = core_of_edge[order]
    e_w = w_of_edge[order]
    e_r = region[order]
    e_idx = idx_in_reg[order]
    e_drel = (dst_local[order] % 128).astype(np.float32)
    e_attr = edge_attr[order]
    k_of = pos_in_bucket // 128              # chunk within bucket
    s_of = pos_in_bucket % 128               # lane within chunk
    assert (k_of < CPR).all()

    gflat = np.zeros((CORES, RCH * NREG * 128), np.int64)   # region-major slots
    dstrel = np.full((CORES, 128, TOTCH), -1.0, np.float32)
    attr17 = np.zeros((CORES, ED + 1, NA * ACOLS), np.float32)
    attr17[:, ED, :] = 1.0               # ones row for the be bias

    # vectorized placement
    g_slot = ((e_r * NW + e_w) * CPR + k_of) * 128 + s_of    # region-major
    c_cons = (e_w * NREG + e_r) * CPR + k_of                 # consumption chunk
    gflat[e_core, g_slot] = e_idx                            # unique slots
    dstrel[e_core, s_of, c_cons] = e_drel
    acol = c_cons * 128 + s_of
    for k in range(ED):
        attr17[e_core, k, acol] = e_attr[:, k]

    gwrap = np.zeros((CORES, 128, GIDXC), np.int16)
    for c in range(CORES):
        gw = gflat[c].reshape(-1, 16).T.astype(np.int16)     # [16, TOT/16]
        gwrap[c] = np.tile(gw, (8, 1))
    attr17 = attr17.astype(bf16)

    # x: replicated storage (quarter-major) + per-core node-major local panel
    x_full = np.zeros((HROWS, D), np.float32)
    x_locN = np.zeros((CORES, 128, NLOC), np.float32)
    for c in range(CORES):
        xc = x[c * NPC:(c + 1) * NPC]
        xp = np.zeros((NLOC, D), np.float32)
        xp[:NPC] = xc
        for q in range(NREG):
            x_full[QOFF[q] + c * QN[q]: QOFF[q] + (c + 1) * QN[q]] = (
                xp[QSTART[q] * 128: QSTART[q] * 128 + QN[q]])
        # node i at partition i%128, cols (i//128)*128 : +128
        x_locN[c] = xp.reshape(NW, 128, D).transpose(1, 0, 2).reshape(128, NLOC)
    x_full = x_full.astype(bf16)
    x_locN = x_locN.astype(bf16)

    # pooling: glocal[p, w] = batch[local node w*128+p] - gbase (pad -> -1)
    glocal = np.full((CORES, 128, NW), -1, np.float32)
    pool_rows = np.zeros((CORES, 128, 2), np.int32)
    for c in range(CORES):
        bb = batch[c * NPC:(c + 1) * NPC]
        gb = int(bb[0])
        span = int(bb[-1] - bb[0])
        assert span < GLW, f"graph span {span} exceeds {GLW}"
        gl = np.full(NLOC, -1, np.int64)
        gl[:NPC] = bb - gb
        glocal[c] = gl.reshape(NW, 128).T.astype(np.float32)
        pool_rows[c, :, 0] = gb + np.arange(128)
        pool_rows[c, :, 1] = gb + 128 + np.arange(128)
    pool_rows = np.clip(pool_rows, 0, G + GLW - 1).astype(np.int32)

    weights = dict(
        WeT=np.ascontiguousarray(np.asarray(We, np.float32)).astype(bf16),
        beb=np.asarray(be, np.float32).astype(bf16),
        W1f=W1f.astype(bf16), W2=np.asarray(W2, np.float32).astype(bf16),
        b1f=b1f.astype(np.float32), b2=np.asarray(b2, np.float32),
        Wc1=np.asarray(Wc1, np.float32).astype(bf16),
        Wc2=np.asarray(Wc2, np.float32).astype(bf16),
        bc1=np.asarray(bc1, np.float32), bc2=np.asarray(bc2, np.float32),
    )
    aux = dict(
        iota=np.tile(np.arange(128, dtype=np.float32),
                     (128, 1)).astype(bf16),
        iota2=np.tile(np.arange(GLW, dtype=np.float32),
                      (128, 1)).astype(bf16),
        ident=np.eye(128, dtype=np.float32).astype(bf16),
    )
    return dict(CPR=CPR, gidx=gwrap, dstrel=dstrel, attr17=attr17,
                x_full=x_full, x_locN=x_locN, glocal=glocal,
                pool_rows=pool_rows, weights=weights, aux=aux)


# ---------------- device program ----------------

def build_program(CPR):
    nc = bacc.Bacc("TRN2", target_bir_lowering=False, debug=False,
                   num_devices=CORES, num_swdge_queues=4)
    TOTCH = NW * NREG * CPR
    RCH = NW * CPR
    GIDXC = TOTCH * 8
    NA = (TOTCH + ASEG - 1) // ASEG
    NA4 = NA * ACOLS
    SEGW = max(1, 20 // CPR)              # windows per gather segment
    NSEG = (NW + SEGW - 1) // SEGW

    t_xfull = nc.dram_tensor("x_full", [HROWS, D], BF16, kind="ExternalInput")
    t_xlocN = nc.dram_tensor("x_locN", [128, NLOC], BF16, kind="ExternalInput")
    t_gidx = nc.dram_tensor("gidx", [128, GIDXC], I16, kind="ExternalInput")
    t_dstrel = nc.dram_tensor("dstrel", [128, TOTCH], F32, kind="ExternalInput")
    t_attr17 = nc.dram_tensor("attr17", [ED + 1, NA * ACOLS], BF16,
                              kind="ExternalInput")
    t_glocal = nc.dram_tensor("glocal", [128, NW], F32, kind="ExternalInput")
    t_prows = nc.dram_tensor("pool_rows", [128, 2], I32, kind="ExternalInput")
    t_iota = nc.dram_tensor("iota", [128, 128], BF16, kind="ExternalInput")
    t_iota2 = nc.dram_tensor("iota2", [128, GLW], BF16, kind="ExternalInput")
    t_ident = nc.dram_tensor("ident", [128, 128], BF16, kind="ExternalInput")
    t_WeT = nc.dram_tensor("WeT", [L, ED, D], BF16, kind="ExternalInput")
    t_beb = nc.dram_tensor("beb", [L, D], BF16, kind="ExternalInput")
    t_W1f = nc.dram_tensor("W1f", [L, D, D], BF16, kind="ExternalInput")
    t_W2 = nc.dram_tensor("W2", [L, D, D], BF16, kind="ExternalInput")
    t_b1f = nc.dram_tensor("b1f", [L, D], F32, kind="ExternalInput")
    t_b2 = nc.dram_tensor("b2", [L, D], F32, kind="ExternalInput")
    t_Wc1 = nc.dram_tensor("Wc1", [D, D], BF16, kind="ExternalInput")
    t_Wc2 = nc.dram_tensor("Wc2", [D, C], BF16, kind="ExternalInput")
    t_bc1 = nc.dram_tensor("bc1", [D], F32, kind="ExternalInput")
    t_bc2 = nc.dram_tensor("bc2", [C], F32, kind="ExternalInput")

    t_out = nc.dram_tensor("out", [C, G], F32, kind="ExternalOutput")

    GROWS = G + GLW

    with tile.TileContext(nc) as tc:
        with (
            tc.tile_pool(name="persist", bufs=1) as pp,
            tc.tile_pool(name="gbuf", bufs=3) as gp,
            tc.tile_pool(name="gidxp", bufs=3) as gip,
            tc.tile_pool(name="attr", bufs=3) as ap_,
            tc.tile_pool(name="mtile", bufs=4) as mp_,
            tc.tile_pool(name="small", bufs=4) as sp,
            tc.tile_pool(name="nodes", bufs=1) as np1,
            tc.tile_pool(name="pse", bufs=3, space="PSUM") as pse,
            tc.tile_pool(name="psa", bufs=2, space="PSUM") as psa,
            tc.tile_pool(name="psn", bufs=2, space="PSUM") as psn,
            tc.tile_pool(name="dram", bufs=1, space="DRAM") as dp,
        ):
            # ---- persistent tiles ----
            h_pan = pp.tile([128, NLOC], BF16, tag="h")          # node-major
            nc.sync.dma_start(h_pan[:], t_xlocN[:])
            dstrel_t = pp.tile([128, TOTCH], F32, tag="dstrel")
            nc.sync.dma_start(dstrel_t[:], t_dstrel[:])
            glocal_t = pp.tile([128, NW], F32, tag="glocal")
            nc.sync.dma_start(glocal_t[:], t_glocal[:])
            prows_t = pp.tile([128, 2], I32, tag="prows")
            nc.sync.dma_start(prows_t[:], t_prows[:])
            iota_t = pp.tile([128, 128], BF16, tag="iota")
            nc.sync.dma_start(iota_t[:], t_iota[:])
            iota2_t = pp.tile([128, GLW], BF16, tag="iota2")
            nc.sync.dma_start(iota2_t[:], t_iota2[:])
            ident = pp.tile([128, 128], BF16, tag="ident")
            nc.sync.dma_start(ident[:], t_ident[:])

            biases = pp.tile([128, 8], F32, tag="biases")
            for l in range(L):
                nc.sync.dma_start(biases[:, 2 * l:2 * l + 1], t_b1f[l, :, None])
                nc.sync.dma_start(biases[:, 2 * l + 1:2 * l + 2], t_b2[l, :, None])
            nc.sync.dma_start(biases[:, 6:7], t_bc1[:, None])
            nc.sync.dma_start(biases[:C, 7:8], t_bc2[:, None])

            WCOLS = 3 * L * D + D + C
            wts = pp.tile([128, WCOLS], BF16, tag="wts")
            nc.vector.memset(wts[:], 0.0)
            for l in range(L):
                nc.sync.dma_start(wts[:ED, 3 * l * D:3 * l * D + D], t_WeT[l])
                nc.sync.dma_start(wts[ED:ED + 1, 3 * l * D:3 * l * D + D],
                                  t_beb[l, None, :])
                nc.sync.dma_start(wts[:, 3 * l * D + D:3 * l * D + 2 * D], t_W1f[l])
                nc.sync.dma_start(wts[:, 3 * l * D + 2 * D:3 * l * D + 3 * D], t_W2[l])
            nc.sync.dma_start(wts[:, 3 * L * D:3 * L * D + D], t_Wc1[:])
            nc.sync.dma_start(wts[:, 3 * L * D + D:3 * L * D + D + C], t_Wc2[:])

            agg = pp.tile([128, NLOC], BF16, tag="agg")          # node-major

            ag_in = []
            ag_out = []
            for q in range(NREG):
                agi = dp.tile([QN[q], D], BF16, tag=f"ag_in{q}",
                              name=f"ag_in{q}")
                ag_in.append(agi)
                bufs = []
                for i in range(2):
                    ago = dp.tile([QROWS[q], D], BF16, tag=f"ag_out{q}_{i}",
                                  name=f"ag_out{q}_{i}")
                    bufs.append(ago)
                ag_out.append(bufs)
            pool_dram = dp.tile([GROWS, D], F32, tag="pool_dram")
            pool_red = dp.tile([GROWS, D], F32, tag="pool_red")

            def edge_layer(l, h_src_of):
                wcol = 3 * l * D
                gtiles = {}     # (r, seg) -> tile

                def emit_gather(r, seg):
                    if seg >= NSEG or DEBUG_NO_GATHER:
                        return
                    ws = seg * SEGW
                    we = min(ws + SEGW, NW)
                    nch = (we - ws) * CPR
                    gt = gp.tile([128, SEGW * CPR, 128], BF16, tag=f"g{r}",
                                 name=f"g{r}")
                    c0 = r * RCH + ws * CPR
                    gi = gip.tile([128, 8 * SEGW * CPR], I16, tag=f"gi{r}",
                                  name=f"gi{r}")
                    nc.sync.dma_start(gi[:, :8 * nch],
                                      t_gidx[:, 8 * c0: 8 * (c0 + nch)])
                    nc.gpsimd.dma_gather(
                        gt[:, :nch, :],
                        h_src_of(r),
                        gi[:, :8 * nch],
                        nch * 128, nch * 128, D,
                        queue_num=r, single_packet=False,
                    )
                    gtiles[(r, seg)] = gt

                atiles = {}

                def emit_attr(seg):
                    if seg >= NA or seg in atiles:
                        return
                    at = ap_.tile([ED + 1, ACOLS], BF16, tag="attr17",
                                  name="at")
                    eng = nc.sync if seg % 2 == 0 else nc.scalar
                    eng.dma_start(at[:],
                                  t_attr17[:, seg * ACOLS:(seg + 1) * ACOLS])
                    atiles[seg] = at

                for r in range(NREG):
                    emit_gather(r, 0)
                for r in range(NREG):
                    emit_gather(r, 1)
                emit_attr(0)
                emit_attr(1)

                wpsums = {}
                pending = None      # (w, grp_with_cc, m, first, last)

                def emit_scatter(item):
                    pw, grp_cc, m, first, last = item
                    for j, cc in enumerate(grp_cc):
                        if DEBUG_NO_SCATTER and not (first and j == 0 or
                                                     last and
                                                     j == len(grp_cc) - 1):
                            continue
                        oh = sp.tile([128, 128], BF16, tag="oh")
                        nc.vector.tensor_scalar(
                            oh[:], iota_t[:], dstrel_t[:, cc:cc + 1], None,
                            op0=mybir.AluOpType.is_equal)
                        nc.tensor.matmul(
                            wpsums[pw][:], oh[:], m[:, j, :],
                            start=(first and j == 0),
                            stop=(last and j == len(grp_cc) - 1))

                def post_window(pw):
                    nc.vector.tensor_copy(agg[:, pw * 128:(pw + 1) * 128],
                                          wpsums.pop(pw)[:])
                    q = W2Q[pw]
                    span_end = (pw == QEND[q]) or ((pw - QSTART[q]) % 4 == 3)
                    if span_end:
                        o = (QSTART[q] + ((pw - QSTART[q]) // 4) * 4) * 128
                        wd = (pw + 1) * 128 - o
                        nw_span = wd // 128
                        nc.vector.tensor_add(agg[:, o:o + wd],
                                             h_pan[:, o:o + wd],
                                             agg[:, o:o + wd])
                        zsp = np1.tile([128, 512], BF16, tag="zsp", bufs=2)
                        for i in range(nw_span):
                            tp = psn.tile([128, 128], BF16, tag="nps", name="tp")
                            nc.tensor.transpose(
                                tp[:], agg[:, o + i * 128:o + (i + 1) * 128],
                                ident[:])
                            nc.vector.tensor_copy(
                                zsp[:, i * 128:(i + 1) * 128], tp[:])
                        ps = psn.tile([128, 512], F32, tag="nps")
                        nc.tensor.matmul(ps[:, :wd],
                                         wts[:, 3 * l * D + D:3 * l * D + 2 * D],
                                         zsp[:, :wd], start=True, stop=True)
                        nc.scalar.activation(zsp[:, :wd], ps[:, :wd],
                                             mybir.ActivationFunctionType.Relu,
                                             bias=biases[:, 2 * l:2 * l + 1])
                        ps2 = psn.tile([128, 512], F32, tag="nps")
                        nc.tensor.matmul(ps2[:, :wd],
                                         wts[:, 3 * l * D + 2 * D:3 * l * D + 3 * D],
                                         zsp[:, :wd], start=True, stop=True)
                        nc.scalar.activation(zsp[:, :wd], ps2[:, :wd],
                                             mybir.ActivationFunctionType.Relu,
                                             bias=biases[:, 2 * l + 1:2 * l + 2])
                        for i in range(nw_span):
                            tp = psn.tile([128, 128], BF16, tag="nps", name="tp")
                            nc.tensor.transpose(
                                tp[:], zsp[:, i * 128:(i + 1) * 128], ident[:])
                            nc.vector.tensor_copy(
                                h_pan[:, o + i * 128:o + (i + 1) * 128], tp[:])
                        if l == L - 1:
                            for wi in range(o // 128, pw + 1):
                                oh0 = sp.tile([128, 128], BF16, tag="oh")
                                nc.vector.tensor_scalar(
                                    oh0[:], iota2_t[:, :128],
                                    glocal_t[:, wi:wi + 1], None,
                                    op0=mybir.AluOpType.is_equal)
                                nc.tensor.matmul(
                                    pps[:, 0, :], oh0[:],
                                    h_pan[:, wi * 128:(wi + 1) * 128],
                                    start=(wi == 0), stop=False)
                                oh1 = sp.tile([128, 128], BF16, tag="oh")
                                nc.vector.tensor_scalar(
                                    oh1[:], iota2_t[:, 128:],
                                    glocal_t[:, wi:wi + 1], None,
                                    op0=mybir.AluOpType.is_equal)
                                nc.tensor.matmul(
                                    pps[:, 1, :], oh1[:],
                                    h_pan[:, wi * 128:(wi + 1) * 128],
                                    start=False, stop=(wi == NW - 1))
                    if pw == QEND[q] and l < L - 1:
                        nc.sync.dma_start(
                            ag_in[q][:].rearrange("(b p) d -> p b d", p=128),
                            h_pan[:, QSTART[q] * 128:(QEND[q] + 1) * 128]
                            .rearrange("p (b d) -> p b d", d=D))
                        if not DEBUG_NO_COLLECTIVES:
                            nc.gpsimd.collective_compute(
                                "AllGather", mybir.AluOpType.bypass,
                                ins=[ag_in[q].opt()],
                                outs=[ag_out[q][l % 2].opt()],
                                replica_groups=[list(range(CORES))])

                nch_w = NREG * CPR
                gctr = 0
                for w in range(NW):
                    seg = w // SEGW
                    if w % SEGW == 0:
                        for r in range(NREG):
                            emit_gather(r, seg + 2)
                    for a in range(((w + 1) * NREG * CPR - 1) // ASEG + 2):
                        emit_attr(a)
                    wpsums[w] = psa.tile([128, D], F32, tag="wpsum",
                                         name="wpsum")
                    chunks = [(r, k) for r in range(NREG) for k in range(CPR)]
                    for g0 in range(0, nch_w, SUB):
                        grp = chunks[g0:g0 + SUB]
                        ng = len(grp)
                        eps = pse.tile([128, SUB * D], F32, tag="eps")
                        for j, (r, k) in enumerate(grp):
                            cc = (w * NREG + r) * CPR + k
                            a_seg, a_j = cc // ASEG, cc % ASEG
                            if DEBUG_NO_EMM and j > 0:
                                continue
                            nc.tensor.matmul(
                                eps[:, j * D:(j + 1) * D],
                                atiles[a_seg][:, a_j * 128:(a_j + 1) * 128],
                                wts[:ED + 1, wcol:wcol + D],
                                start=(j == 0),
                                stop=False)
                        for j, (r, k) in enumerate(grp):
                            off = (w - seg * SEGW) * CPR + k
                            hsrc = (iota_t if DEBUG_NO_GATHER
                                    else gtiles[(r, seg)][:, off, :])
                            nc.tensor.matmul(
                                eps[:, j * D:(j + 1) * D],
                                ident[:], hsrc[:],
                                start=False, stop=(j == ng - 1))
                        m = mp_.tile([128, SUB, 128], BF16, tag="m")
                        if gctr % 2 == 0:
                            nc.scalar.activation(
                                m[:, :ng, :].rearrange("p a d -> p (a d)"),
                                eps[:, :ng * D],
                                mybir.ActivationFunctionType.Relu)
                        else:
                            nc.vector.tensor_scalar(
                                m[:, :ng, :].rearrange("p a d -> p (a d)"),
                                eps[:, :ng * D], 0.0, None,
                                op0=mybir.AluOpType.max)
                        gctr += 1
                        if pending is not None:
                            emit_scatter(pending)
                            if pending[4]:          # closed a window
                                post_window(pending[0])
                        cc_list = [(w * NREG + r) * CPR + k for (r, k) in grp]
                        pending = (w, cc_list, m, g0 == 0,
                                   g0 + SUB >= nch_w)
                if pending is not None:
                    emit_scatter(pending)
                    if pending[4]:
                        post_window(pending[0])
                    pending = None

            # ---------------- layers ----------------
            W2Q = []
            for q in range(NREG):
                W2Q += [q] * QWIN[q]
            pps = psa.tile([128, 2, D], F32, tag="pps", bufs=1)
            for l in range(L):
                if l == 0:
                    def h_src_of(r):
                        return t_xfull[QOFF[r]:QOFF[r] + QROWS[r], :]
                else:
                    def h_src_of(r, _l=l):
                        return ag_out[r][(_l - 1) % 2][:]
                edge_layer(l, h_src_of)

            # ---------------- pooling epilogue ----------------
            pool_sb = np1.tile([128, 2, D], F32, tag="pool_sb")
            nc.scalar.copy(pool_sb[:, 0, :], pps[:, 0, :])
            nc.scalar.copy(pool_sb[:, 1, :], pps[:, 1, :])

            # zero the global pooled buffer, then place partials at gbase rows
            zt = np1.tile([128, (GROWS // 128) * D], F32, tag="zt")
            nc.vector.memset(zt[:], 0.0)
            nc.sync.dma_start(
                pool_dram[:].rearrange("(a p) d -> p a d", p=128),
                zt[:].rearrange("p (a d) -> p a d", d=D))
            for i in range(2):
                nc.gpsimd.indirect_dma_start(
                    out=pool_dram[:], out_offset=bass.IndirectOffsetOnAxis(
                        ap=prows_t[:, i:i + 1], axis=0),
                    in_=pool_sb[:, i, :], in_offset=None)
            if DEBUG_NO_COLLECTIVES:
                nc.vector.memset(zt[:, :D], 1.0)
                nc.sync.dma_start(
                    pool_red[:].rearrange("(a p) d -> p a d", p=128),
                    zt[:].rearrange("p (a d) -> p a d", d=D))
            else:
                nc.gpsimd.collective_compute(
                    "AllReduce", mybir.AluOpType.add,
                    ins=[pool_dram.opt()], outs=[pool_red.opt()],
                    replica_groups=[list(range(CORES))])

            # ---------------- classifier ----------------
            prows_n = np1.tile([128, G // 128, D], F32, tag="prows_n")
            nc.sync.dma_start(
                prows_n[:], pool_red[:G, :].rearrange("(b p) d -> p b d", p=128))
            prows_bf = np1.tile([128, G // 128, D], BF16, tag="prows_bf")
            nc.vector.tensor_copy(
                prows_bf[:].rearrange("p a d -> p (a d)"),
                prows_n[:].rearrange("p a d -> p (a d)"))
            pooled_bf = np1.tile([128, G], BF16, tag="pooled_bf")   # D-major
            for b in range(G // 128):
                tp = psn.tile([128, 128], BF16, tag="nps", name="tp")
                nc.tensor.transpose(tp[:], prows_bf[:, b, :], ident[:])
                nc.vector.tensor_copy(pooled_bf[:, b * 128:(b + 1) * 128], tp[:])
            q1 = np1.tile([128, G], BF16, tag="q1")
            for o in range(0, G, 512):
                wd = min(512, G - o)
                ps = psn.tile([128, 512], F32, tag="nps")
                nc.tensor.matmul(ps[:, :wd], wts[:, 3 * L * D:3 * L * D + D],
                                 pooled_bf[:, o:o + wd], start=True, stop=True)
                nc.scalar.activation(q1[:, o:o + wd], ps[:, :wd],
                                     mybir.ActivationFunctionType.Relu,
                                     bias=biases[:, 6:7])
            outt = np1.tile([C, G], F32, tag="outt")
            for o in range(0, G, 512):
                wd = min(512, G - o)
                ps = psn.tile([128, 512], F32, tag="nps")
                nc.tensor.matmul(ps[:C, :wd], wts[:, 3 * L * D + D:3 * L * D + D + C],
                                 q1[:, o:o + wd], start=True, stop=True)
                nc.scalar.activation(outt[:, o:o + wd], ps[:C, :wd],
                                     mybir.ActivationFunctionType.Identity,
                                     bias=biases[:C, 7:8])
            nc.sync.dma_start(t_out[:], outt[:])

    nc.compile()
    return nc


_PROGRAM_CACHE = {}


def _get_program(CPR):
    if CPR not in _PROGRAM_CACHE:
        _PROGRAM_CACHE[CPR] = build_program(CPR)
    return _PROGRAM_CACHE[CPR]


def make_in_maps(prep):
    w = prep["weights"]
    a = prep["aux"]
    in_maps = []
    for c in range(CORES):
        in_maps.append({
            "x_full": prep["x_full"],
            "x_locN": np.ascontiguousarray(prep["x_locN"][c]),
            "gidx": np.ascontiguousarray(prep["gidx"][c]),
            "dstrel": np.ascontiguousarray(prep["dstrel"][c]),
            "attr17": np.ascontiguousarray(prep["attr17"][c]),
            "glocal": np.ascontiguousarray(prep["glocal"][c]),
            "pool_rows": np.ascontiguousarray(prep["pool_rows"][c]),
            "iota": a["iota"], "iota2": a["iota2"], "ident": a["ident"],
            "WeT": w["WeT"], "beb": w["beb"], "W1f": w["W1f"], "W2": w["W2"],
            "b1f": w["b1f"], "b2": w["b2"],
            "Wc1": w["Wc1"], "Wc2": w["Wc2"], "bc1": w["bc1"], "bc2": w["bc2"],
        })
    return in_maps


def postprocess(out):
    return np.ascontiguousarray(out.T.astype(np.float32))


def kernel(**inputs):
    prep = prepare(**{k: np.asarray(v) for k, v in inputs.items()})
    nc = _get_program(prep["CPR"])
    res = bass_utils.run_bass_kernel_spmd(nc, make_in_maps(prep),
                                          core_ids=list(range(CORES)))
    return postprocess(res.results[0]["out"])
